# Initial kernel scaffold
#
"""ClusterMemory forward (logits = inputs @ features.T / temp) + sequential
EMA scatter update of the feature bank, distributed across 8 trn2 NeuronCores.

Sharding: features [65536, 256] row-sharded 8192 rows/core. Each core computes
its [512, 8192] logits column block and applies the EMA updates whose target
rows it owns; host concatenates the shards.

The order-dependent EMA scan is restructured as "rounds": round r applies the
r-th occurrence of every distinct target row. Rows within a round are unique,
so each round is one vectorized EMA+renorm over <=128 gathered rows. Per-core
row indices / per-round coefficients are data (SPMD-safe), prepared on host.
"""

import os
from contextlib import ExitStack

import numpy as np

import concourse.bass as bass
import concourse.tile as tile
from concourse import mybir
from concourse.bass_utils import run_bass_kernel_spmd

B = 512  # batch
D = 256  # feature dim
NS = 65536  # memory bank rows
NCORES = 8
RPC = NS // NCORES  # rows per core = 8192
TEMP = 0.05
MOMENTUM = 0.2
EPS = 1e-12
P = 128

JB = 512  # matmul moving free dim (j chunk)
NJC = RPC // JB  # 16 j-chunks per core
JH = 8  # j-chunks per psum group (8 psum banks)

LAST_RESULTS = None  # BassKernelResults of the most recent run (for test.py)


def _build_schedule(targets: np.ndarray):
    """Per-core update plan.

    Returns (uidx[NCORES, S], A[NCORES, R, S], occ[NCORES]) where
    uidx = local row per slot, A[r, s] = momentum if slot s active in round r
    else 1.0, and occ[c] = list of (r, slot, batch_i) active entries.
    """
    per_core = [[] for _ in range(NCORES)]
    for i, y in enumerate(targets.tolist()):
        c = y // RPC
        per_core[c].append((i, y - c * RPC))

    plans = []
    for c in range(NCORES):
        slot_of = {}
        counts = {}
        entries = []  # (r, slot, batch_i)
        for i, ly in per_core[c]:
            if ly not in slot_of:
                slot_of[ly] = len(slot_of)
            r = counts.get(ly, 0)
            counts[ly] = r + 1
            entries.append((r, slot_of[ly], i))
        plans.append((slot_of, entries))

    u_max = max(len(p[0]) for p in plans)
    n_rounds = max([e[0] for p in plans for e in p[1]], default=0) + 1
    n_rounds = max(n_rounds, 1)
    n_tiles = max(1, -(-u_max // P))
    S = n_tiles * P

    uidx = np.zeros((NCORES, S), dtype=np.int32)
    A = np.ones((NCORES, n_rounds, S), dtype=np.float32)
    occ = []
    for c in range(NCORES):
        slot_of, entries = plans[c]
        used = set(slot_of.keys())
        pad_row = 0
        while pad_row in used:
            pad_row += 1
        uidx[c, :] = pad_row
        for ly, s in slot_of.items():
            uidx[c, s] = ly
        for r, s, _ in entries:
            A[c, r, s] = MOMENTUM
        occ.append(entries)
    return uidx, A, occ, n_rounds, n_tiles


def _build_program(n_rounds: int, n_tiles: int, mm_mode: str):
    dt = mybir.dt
    S = n_tiles * P
    mm_in_dt = dt.bfloat16 if mm_mode == "bf16" else dt.float32

    nc = bass.Bass("TRN2", debug=False, num_devices=NCORES)
    featT = nc.dram_tensor("featT", [2, P, RPC], mm_in_dt, kind="ExternalInput")
    feat = nc.dram_tensor("feat", [RPC, D], dt.float32, kind="ExternalInput")
    inT = nc.dram_tensor("inT", [2, P, B], mm_in_dt, kind="ExternalInput")
    uidx = nc.dram_tensor("uidx", [S, 1], dt.int32, kind="ExternalInput")
    ema_a = nc.dram_tensor("ema_a", [n_rounds, S, 1], dt.float32, kind="ExternalInput")
    ema_x = nc.dram_tensor("ema_x", [n_rounds, S, D], dt.float32, kind="ExternalInput")
    logits = nc.dram_tensor("logits", [B, RPC], dt.float32, kind="ExternalOutput")
    newfeat = nc.dram_tensor("newfeat", [RPC, D], dt.float32, kind="ExternalOutput")

    featT_ap, feat_ap, inT_ap = featT.ap(), feat.ap(), inT.ap()
    uidx_ap, ema_a_ap, ema_x_ap = uidx.ap(), ema_a.ap(), ema_x.ap()
    logits_ap, newfeat_ap = logits.ap(), newfeat.ap()

    inv_temp = float(np.float32(1.0) / np.float32(TEMP))

    def mm_cast(ap):
        return ap.bitcast(dt.float32r) if mm_mode == "f32r" else ap

    with ExitStack() as ctx:
        tc = ctx.enter_context(tile.TileContext(nc))

        weights = ctx.enter_context(tc.tile_pool(name="weights", bufs=1))
        psums = ctx.enter_context(tc.tile_pool(name="psums", bufs=8, space="PSUM"))
        stages = ctx.enter_context(tc.tile_pool(name="stages", bufs=2))
        ema = ctx.enter_context(tc.tile_pool(name="ema", bufs=2))
        emag = ctx.enter_context(tc.tile_pool(name="emag", bufs=2 * n_tiles))

        # Bulk copy of the untouched bank shard; updated rows overwritten by
        # the indirect scatter below (WAW dep on newfeat orders the two).
        nc.gpsimd.dma_start(out=newfeat_ap[:, :], in_=feat_ap[:, :])

        # Resident matmul operands.
        ft = [weights.tile([P, RPC], mm_in_dt, tag=f"ft{k}") for k in range(2)]
        it = [weights.tile([P, B], mm_in_dt, tag=f"it{k}") for k in range(2)]
        for k in range(2):
            nc.sync.dma_start(out=ft[k][:], in_=featT_ap[k])
            nc.sync.dma_start(out=it[k][:], in_=inT_ap[k])

        # ---- EMA update path (tiny; overlaps with matmuls) ----
        for t in range(n_tiles):
            uidx_sb = ema.tile([P, 1], dt.int32, tag="uidx")
            nc.gpsimd.dma_start(out=uidx_sb[:], in_=uidx_ap[t * P:(t + 1) * P, :])
            g = emag.tile([P, D], dt.float32, tag="g")
            nc.gpsimd.indirect_dma_start(
                out=g[:],
                out_offset=None,
                in_=feat_ap[:, :],
                in_offset=bass.IndirectOffsetOnAxis(ap=uidx_sb[:, :1], axis=0),
            )
            for r in range(n_rounds):
                a_sb = ema.tile([P, 1], dt.float32, tag="a")
                x_sb = ema.tile([P, D], dt.float32, tag="x")
                nc.gpsimd.dma_start(out=a_sb[:], in_=ema_a_ap[r, t * P:(t + 1) * P, :])
                nc.gpsimd.dma_start(out=x_sb[:], in_=ema_x_ap[r, t * P:(t + 1) * P, :])
                g2 = emag.tile([P, D], dt.float32, tag="g")
                # g2 = g * a + x   (a per-partition; x pre-scaled by (1-m))
                nc.vector.scalar_tensor_tensor(
                    out=g2[:], in0=g[:], scalar=a_sb[:], in1=x_sb[:],
                    op0=mybir.AluOpType.mult, op1=mybir.AluOpType.add,
                )
                sq = ema.tile([P, D], dt.float32, tag="sq")
                ss = ema.tile([P, 1], dt.float32, tag="ss")
                nc.vector.tensor_tensor_reduce(
                    out=sq[:], in0=g2[:], in1=g2[:], scale=1.0, scalar=EPS,
                    op0=mybir.AluOpType.mult, op1=mybir.AluOpType.add,
                    accum_out=ss[:],
                )
                nc.scalar.activation(
                    out=ss[:], in_=ss[:], func=mybir.ActivationFunctionType.Sqrt,
                )
                nc.vector.reciprocal(out=ss[:], in_=ss[:])
                g = emag.tile([P, D], dt.float32, tag="g")
                nc.vector.tensor_scalar_mul(out=g[:], in0=g2[:], scalar1=ss[:])
            nc.gpsimd.indirect_dma_start(
                out=newfeat_ap[:, :],
                out_offset=bass.IndirectOffsetOnAxis(ap=uidx_sb[:, :1], axis=0),
                in_=g[:],
                in_offset=None,
            )

        # ---- logits matmul: out[b, j] = sum_d inT[d, b] * featT[d, j] ----
        for bt in range(B // P):
            for jh in range(NJC // JH):
                ps = [psums.tile([P, JB], dt.float32, tag="ps") for _ in range(JH)]
                for k in range(2):
                    lhsT = mm_cast(it[k][:, bt * P:(bt + 1) * P])
                    for jc in range(JH):
                        j0 = (jh * JH + jc) * JB
                        nc.tensor.matmul(
                            out=ps[jc][:],
                            lhsT=lhsT,
                            rhs=mm_cast(ft[k][:, j0:j0 + JB]),
                            start=(k == 0),
                            stop=(k == 1),
                        )
                stage = stages.tile([P, JH * JB], dt.float32, tag="stage")
                for jc in range(JH):
                    nc.vector.tensor_scalar_mul(
                        out=stage[:, jc * JB:(jc + 1) * JB], in0=ps[jc][:],
                        scalar1=inv_temp,
                    )
                nc.scalar.dma_start(
                    out=logits_ap[bt * P:(bt + 1) * P, jh * JH * JB:(jh + 1) * JH * JB],
                    in_=stage[:],
                )
    return nc


def kernel(**inputs):
    global LAST_RESULTS
    x = np.ascontiguousarray(np.asarray(inputs["inputs"], dtype=np.float32))
    targets = np.asarray(inputs["targets"]).astype(np.int64)
    features = np.ascontiguousarray(np.asarray(inputs["features"], dtype=np.float32))

    mm_mode = os.environ.get("BASS_KERNEL_MM", "f32r")
    uidx, A, occ, n_rounds, n_tiles = _build_schedule(targets)
    S = n_tiles * P

    nc = _build_program(n_rounds, n_tiles, mm_mode)

    np_mm = np.float32 if mm_mode != "bf16" else None
    if mm_mode == "bf16":
        import ml_dtypes
        np_mm = ml_dtypes.bfloat16

    inT_full = np.ascontiguousarray(x.T).reshape(2, P, B).astype(np_mm)

    in_maps = []
    for c in range(NCORES):
        shard = features[c * RPC:(c + 1) * RPC]
        featT_c = np.ascontiguousarray(shard.T).reshape(2, P, RPC).astype(np_mm)
        ema_x_c = np.zeros((n_rounds, S, D), dtype=np.float32)
        for r, s, i in occ[c]:
            ema_x_c[r, s, :] = (1.0 - np.float32(MOMENTUM)) * x[i]
        in_maps.append({
            "featT": featT_c,
            "feat": shard,
            "inT": inT_full,
            "uidx": uidx[c].reshape(S, 1),
            "ema_a": A[c].reshape(n_rounds, S, 1),
            "ema_x": ema_x_c,
        })

    res = run_bass_kernel_spmd(nc, in_maps, core_ids=list(range(NCORES)))
    LAST_RESULTS = res

    logits = np.concatenate([r["logits"] for r in res.results], axis=1)
    new_features = np.concatenate([r["newfeat"] for r in res.results], axis=0)
    return logits, new_features


# revision 7
# speedup vs baseline: 64.0955x; 64.0955x over previous
"""ClusterMemory forward (logits = inputs @ features.T / temp) + sequential
EMA scatter update of the feature bank, distributed across 8 trn2 NeuronCores.

Sharding: features [65536, 256] row-sharded 8192 rows/core. Each core computes
its [512, 8192] logits column block and applies the EMA updates whose target
rows it owns; host concatenates the shards.

The order-dependent EMA scan is restructured as "rounds": round r applies the
r-th occurrence of every distinct target row. Rows within a round are unique,
so each round is one vectorized EMA+renorm over <=128 gathered rows. Per-core
row indices / per-round coefficients are data (SPMD-safe), prepared on host.
"""

import os
from contextlib import ExitStack

import numpy as np

import concourse.bass as bass
import concourse.tile as tile
from concourse import bacc, mybir
from concourse.bass_utils import run_bass_kernel_spmd

B = 512  # batch
D = 256  # feature dim
NS = 65536  # memory bank rows
NCORES = 8
RPC = NS // NCORES  # rows per core = 8192
TEMP = 0.05
MOMENTUM = 0.2
EPS = 1e-12
P = 128

JB = 512  # matmul moving free dim (j chunk)
NJC = RPC // JB  # 16 j-chunks per core
JH = 8  # j-chunks per psum group (8 psum banks)

LAST_RESULTS = None  # BassKernelResults of the most recent run (for test.py)


def _build_schedule(targets: np.ndarray):
    """Per-core update plan.

    Returns (uidx[NCORES, S], A[NCORES, R, S], occ[NCORES]) where
    uidx = local row per slot, A[r, s] = momentum if slot s active in round r
    else 1.0, and occ[c] = list of (r, slot, batch_i) active entries.
    """
    per_core = [[] for _ in range(NCORES)]
    for i, y in enumerate(targets.tolist()):
        c = y // RPC
        per_core[c].append((i, y - c * RPC))

    plans = []
    for c in range(NCORES):
        slot_of = {}
        counts = {}
        entries = []  # (r, slot, batch_i)
        for i, ly in per_core[c]:
            if ly not in slot_of:
                slot_of[ly] = len(slot_of)
            r = counts.get(ly, 0)
            counts[ly] = r + 1
            entries.append((r, slot_of[ly], i))
        plans.append((slot_of, entries))

    u_max = max(len(p[0]) for p in plans)
    n_rounds = max([e[0] for p in plans for e in p[1]], default=0) + 1
    n_rounds = max(n_rounds, 1)
    n_tiles = max(1, -(-u_max // P))
    S = n_tiles * P

    uidx = np.zeros((NCORES, S), dtype=np.int32)
    A = np.ones((NCORES, n_rounds, S), dtype=np.float32)
    occ = []
    for c in range(NCORES):
        slot_of, entries = plans[c]
        used = set(slot_of.keys())
        pad_row = 0
        while pad_row in used:
            pad_row += 1
        uidx[c, :] = pad_row
        for ly, s in slot_of.items():
            uidx[c, s] = ly
        for r, s, _ in entries:
            A[c, r, s] = MOMENTUM
        occ.append(entries)
    return uidx, A, occ, n_rounds, n_tiles


def _emit_body(nc, pools, aps, n_rounds, n_tiles, mm_in_dt, rep):
    dt = mybir.dt
    weights, psums, stages, ema, emag = pools
    featT_ap, feat_ap, inT_ap, uidx_ap, ema_a_ap, ema_x_ap, logits_ap, newfeat_ap = aps
    inv_temp = float(np.float32(1.0) / np.float32(TEMP))

    # Bulk copy of the untouched bank shard; updated rows overwritten by the
    # indirect scatter below (WAW dep on newfeat orders the two).
    nc.gpsimd.dma_start(out=newfeat_ap[:, :], in_=feat_ap[:, :])

    # Resident matmul operands.
    ft = [weights.tile([P, RPC], mm_in_dt, name=f"ft{k}_{rep}", tag=f"ft{k}")
          for k in range(2)]
    it = [weights.tile([P, B], mm_in_dt, name=f"it{k}_{rep}", tag=f"it{k}")
          for k in range(2)]
    for k in range(2):
        nc.sync.dma_start(out=ft[k][:], in_=featT_ap[k])
        nc.sync.dma_start(out=it[k][:], in_=inT_ap[k])

    # ---- EMA update path (tiny; overlaps with matmuls) ----
    for t in range(n_tiles):
        uidx_sb = ema.tile([P, 1], dt.int32, name=f"uidx_{rep}_{t}", tag="uidx")
        nc.gpsimd.dma_start(out=uidx_sb[:], in_=uidx_ap[t * P:(t + 1) * P, :])
        g = emag.tile([P, D], dt.float32, name=f"g_{rep}_{t}", tag="g")
        nc.gpsimd.indirect_dma_start(
            out=g[:],
            out_offset=None,
            in_=feat_ap[:, :],
            in_offset=bass.IndirectOffsetOnAxis(ap=uidx_sb[:, :1], axis=0),
        )
        for r in range(n_rounds):
            a_sb = ema.tile([P, 1], dt.float32, name=f"a_{rep}_{t}_{r}", tag="a")
            x_sb = ema.tile([P, D], dt.float32, name=f"x_{rep}_{t}_{r}", tag="x")
            nc.sync.dma_start(out=a_sb[:], in_=ema_a_ap[r, t * P:(t + 1) * P, :])
            nc.sync.dma_start(out=x_sb[:], in_=ema_x_ap[r, t * P:(t + 1) * P, :])
            g2 = emag.tile([P, D], dt.float32, name=f"g2_{rep}_{t}_{r}", tag="g")
            # g2 = g * a + x   (a per-partition; x pre-scaled by (1-m))
            nc.vector.scalar_tensor_tensor(
                out=g2[:], in0=g[:], scalar=a_sb[:], in1=x_sb[:],
                op0=mybir.AluOpType.mult, op1=mybir.AluOpType.add,
            )
            # rsqrt(sum(g2^2)+eps) via ACT only: this environment's runtime
            # crashes on InstTensorTensorReduce / InstReciprocal, so use
            # Square+accum then exp(-0.5*ln(s+eps)).
            sq = ema.tile([P, D], dt.float32, name=f"sq_{rep}_{t}_{r}", tag="sq")
            ss = ema.tile([P, 1], dt.float32, name=f"ss_{rep}_{t}_{r}", tag="ss")
            nc.scalar.activation(
                out=sq[:], in_=g2[:],
                func=mybir.ActivationFunctionType.Square, accum_out=ss[:],
            )
            nc.vector.tensor_scalar_add(out=ss[:], in0=ss[:], scalar1=EPS)
            nc.scalar.activation(
                out=ss[:], in_=ss[:], func=mybir.ActivationFunctionType.Ln,
            )
            nc.scalar.activation(
                out=ss[:], in_=ss[:],
                func=mybir.ActivationFunctionType.Exp, scale=-0.5,
            )
            g = emag.tile([P, D], dt.float32, name=f"gn_{rep}_{t}_{r}", tag="g")
            nc.vector.tensor_scalar_mul(out=g[:], in0=g2[:], scalar1=ss[:])
        nc.gpsimd.indirect_dma_start(
            out=newfeat_ap[:, :],
            out_offset=bass.IndirectOffsetOnAxis(ap=uidx_sb[:, :1], axis=0),
            in_=g[:],
            in_offset=None,
        )

    # ---- logits matmul: out[b, j] = sum_d inT[d, b] * featT[d, j] ----
    for bt in range(B // P):
        for jh in range(NJC // JH):
            ps = [psums.tile([P, JB], dt.float32, name=f"ps{jc}_{rep}", tag="ps")
                  for jc in range(JH)]
            for k in range(2):
                lhsT = it[k][:, bt * P:(bt + 1) * P]
                for jc in range(JH):
                    j0 = (jh * JH + jc) * JB
                    nc.tensor.matmul(
                        out=ps[jc][:],
                        lhsT=lhsT,
                        rhs=ft[k][:, j0:j0 + JB],
                        start=(k == 0),
                        stop=(k == 1),
                    )
            stage = stages.tile([P, JH * JB], dt.float32,
                                name=f"stage_{rep}", tag="stage")
            for jc in range(JH):
                nc.vector.tensor_scalar_mul(
                    out=stage[:, jc * JB:(jc + 1) * JB], in0=ps[jc][:],
                    scalar1=inv_temp,
                )
            nc.scalar.dma_start(
                out=logits_ap[bt * P:(bt + 1) * P, jh * JH * JB:(jh + 1) * JH * JB],
                in_=stage[:],
            )


def _build_program(n_rounds: int, n_tiles: int, mm_mode: str, reps: int = 1):
    dt = mybir.dt
    S = n_tiles * P
    mm_in_dt = {"bf16": dt.bfloat16, "f32r": dt.float32r, "f32": dt.float32}[mm_mode]

    nc = bacc.Bacc("TRN2", debug=False, num_devices=NCORES)
    featT = nc.dram_tensor("featT", [2, P, RPC], mm_in_dt, kind="ExternalInput")
    feat = nc.dram_tensor("feat", [RPC, D], dt.float32, kind="ExternalInput")
    inT = nc.dram_tensor("inT", [2, P, B], mm_in_dt, kind="ExternalInput")
    uidx = nc.dram_tensor("uidx", [S, 1], dt.int32, kind="ExternalInput")
    ema_a = nc.dram_tensor("ema_a", [n_rounds, S, 1], dt.float32, kind="ExternalInput")
    ema_x = nc.dram_tensor("ema_x", [n_rounds, S, D], dt.float32, kind="ExternalInput")
    logits = nc.dram_tensor("logits", [B, RPC], dt.float32, kind="ExternalOutput")
    newfeat = nc.dram_tensor("newfeat", [RPC, D], dt.float32, kind="ExternalOutput")

    aps = (featT.ap(), feat.ap(), inT.ap(), uidx.ap(), ema_a.ap(), ema_x.ap(),
           logits.ap(), newfeat.ap())

    with ExitStack() as ctx:
        tc = ctx.enter_context(tile.TileContext(nc))
        weights = ctx.enter_context(tc.tile_pool(name="weights", bufs=1))
        psums = ctx.enter_context(tc.tile_pool(name="psums", bufs=8, space="PSUM"))
        stages = ctx.enter_context(tc.tile_pool(name="stages", bufs=2))
        ema = ctx.enter_context(tc.tile_pool(name="ema", bufs=2))
        emag = ctx.enter_context(tc.tile_pool(name="emag", bufs=2 * n_tiles))
        pools = (weights, psums, stages, ema, emag)
        # reps>1 repeats the whole body back-to-back for wall-clock timing
        # (amortizes the axon RPC overhead out); reps=1 for real runs.
        for rep in range(reps):
            _emit_body(nc, pools, aps, n_rounds, n_tiles, mm_in_dt, rep)
    nc.compile()
    return nc


def _make_in_maps(x, targets, features, uidx, A, occ, n_rounds, mm_mode):
    S = uidx.shape[1]
    np_mm = np.float32
    if mm_mode == "bf16":
        import ml_dtypes
        np_mm = ml_dtypes.bfloat16
    inT_full = np.ascontiguousarray(x.T).reshape(2, P, B).astype(np_mm)
    in_maps = []
    for c in range(NCORES):
        shard = features[c * RPC:(c + 1) * RPC]
        featT_c = np.ascontiguousarray(shard.T).reshape(2, P, RPC).astype(np_mm)
        ema_x_c = np.zeros((n_rounds, S, D), dtype=np.float32)
        for r, s, i in occ[c]:
            ema_x_c[r, s, :] = (1.0 - np.float32(MOMENTUM)) * x[i]
        in_maps.append({
            "featT": featT_c,
            "feat": shard,
            "inT": inT_full,
            "uidx": uidx[c].reshape(S, 1),
            "ema_a": A[c].reshape(n_rounds, S, 1),
            "ema_x": ema_x_c,
        })
    return in_maps


def kernel(**inputs):
    global LAST_RESULTS
    x = np.ascontiguousarray(np.asarray(inputs["inputs"], dtype=np.float32))
    targets = np.asarray(inputs["targets"]).astype(np.int64)
    features = np.ascontiguousarray(np.asarray(inputs["features"], dtype=np.float32))

    mm_mode = os.environ.get("BASS_KERNEL_MM", "f32r")
    uidx, A, occ, n_rounds, n_tiles = _build_schedule(targets)

    nc = _build_program(n_rounds, n_tiles, mm_mode)
    in_maps = _make_in_maps(x, targets, features, uidx, A, occ, n_rounds, mm_mode)

    res = run_bass_kernel_spmd(nc, in_maps, core_ids=list(range(NCORES)))
    LAST_RESULTS = res

    logits = np.concatenate([r["logits"] for r in res.results], axis=1)
    new_features = np.concatenate([r["newfeat"] for r in res.results], axis=0)
    return logits, new_features
